# revision 1
# baseline (speedup 1.0000x reference)
"""Collaborative attention (nn_CollaborativeAttention) on 8 Trainium2 NeuronCores.

Reference math (B=2, S=2048, D=1024, H=16 heads, head mixing over full DKQ=1024):
    q = h @ Wq.T ; k = h @ Wk.T ; v = h @ Wv.T + bv
    scores[b,h,s,t] = sum_e q[b,s,e] * mixing[h,e] * k[b,t,e] / sqrt(64)
    probs = softmax_t(scores) ; ctx[b,s,:] = concat_h(probs @ v_head)

Sharding: core c handles batch b = c//4 and head group g = c%4 (4 heads each).
No cross-core communication; host slices inputs / concatenates outputs.

All large matmuls run as fp8(e4m3) DoubleRow with a hi/lo split-compensation:
x ~ x_hi + x_lo (both fp8, lo = fp8(x - x_hi)), and x*y is computed with the
three dominant terms hi*hi + lo*hi + hi*lo (the lo*lo term is ~1e-3 relative
and dropped). A DoubleRow matmul contracts two 128-chunks per instruction at
0.5 cycles/row, so 3 instructions per chunk-PAIR replace 2 f16 instructions
per pair: 1.33x PE throughput at ~2e-3 end-to-end error. Tensors are
pre-scaled so the fp8 lo residuals stay above the e4m3 denormal floor:
weights are uploaded as 32*W.T, kt is stored as 32*k, mq as 8*q*mix (mix is
uploaded pre-divided by 4), and exp() absorbs the 256x with scale/256.
The v bias is folded in on the host (softmax rows sum to 1 => ctx = probs@v
+ bv), so the projection contraction stays at 8 chunk-pairs with no ones-row.

Device dataflow (per core):
    kt32[e,t] hi/lo fp8 <- K projection (fp8 3-term), epilogue on gpsimd+DVE
    v[t,dv] f16         <- V projection (fp8 3-term), scaled 1/32 on DVE
    per s-block of 512 queries (Q projection emitted one block ahead so the
    mq hi/lo epilogue hides under the previous block's score matmuls):
      mq[e,s] hi/lo fp8 = (32q psum) * (mix/4)   (hi: DVE/ACT, lo: gpsimd)
      scoresT = 3-term fp8 DoubleRow -> exp((0.125/256)*x) -> expT[t,s] f16
      ctxT_aug = sum_t v_chunk.T @ expT (f16 PSUM accum; row 64 = denominator)
      PE-transpose 128x128 -> multiply by reciprocal(denominator) -> DMA out.
"""

import math

import numpy as np

B, S, D = 2, 2048, 1024
H, DV = 16, 1024
N_CORES = 8
HG = 4  # heads per core
DH = 64  # head dim
P = 128
EC = 8  # e-chunks (1024/128)
DC = 8  # d-chunks (1024/128, no ones row)
NPAIR = 4  # contraction chunk-pairs (1024/256)
NB = 512  # s-block width
SBLK = 4  # number of s blocks
TCH = 16  # t-chunks of 128
SCALE = 1.0 / math.sqrt(D / H)  # 0.125
EXP_SCALE = SCALE / 256.0  # psum holds (8*q*mix)*(32*k) = 256*scores

_CACHE: dict = {}


def build_program():
    """Build the (SPMD, per-core) Bass program."""
    import concourse.bass as bass
    import concourse.mybir as mybir
    from concourse import bacc
    from concourse.tile import TileContext
    from concourse.masks import make_identity

    f32 = mybir.dt.float32
    f16 = mybir.dt.float16
    f8 = mybir.dt.float8e4
    mult = mybir.AluOpType.mult
    sub = mybir.AluOpType.subtract
    Exp = mybir.ActivationFunctionType.Exp
    Copy = mybir.ActivationFunctionType.Copy
    DR = mybir.MatmulPerfMode.DoubleRow

    nc = bacc.Bacc("TRN2", target_bir_lowering=False, debug=True)
    hthi = nc.dram_tensor("hthi", [D, S], f8, kind="ExternalInput")
    htlo = nc.dram_tensor("htlo", [D, S], f8, kind="ExternalInput")
    wqthi = nc.dram_tensor("wqthi", [D, D], f8, kind="ExternalInput")
    wqtlo = nc.dram_tensor("wqtlo", [D, D], f8, kind="ExternalInput")
    wkthi = nc.dram_tensor("wkthi", [D, D], f8, kind="ExternalInput")
    wktlo = nc.dram_tensor("wktlo", [D, D], f8, kind="ExternalInput")
    wvthi = nc.dram_tensor("wvthi", [D, HG * DH], f8, kind="ExternalInput")
    wvtlo = nc.dram_tensor("wvtlo", [D, HG * DH], f8, kind="ExternalInput")
    mix = nc.dram_tensor("mix", [P, EC * HG], f32, kind="ExternalInput")
    ctx_o = nc.dram_tensor("ctx", [S, HG * DH], f32, kind="ExternalOutput")

    hthi_v = hthi.rearrange("(c p) s -> p c s", p=P)  # [128, 8, 2048]
    htlo_v = htlo.rearrange("(c p) s -> p c s", p=P)
    wqthi_v = wqthi.rearrange("(c p) e -> p c e", p=P)  # [128, 8, 1024]
    wqtlo_v = wqtlo.rearrange("(c p) e -> p c e", p=P)
    wkthi_v = wkthi.rearrange("(c p) e -> p c e", p=P)
    wktlo_v = wktlo.rearrange("(c p) e -> p c e", p=P)
    wvthi_v = wvthi.rearrange("(c p) n -> p c n", p=P)  # [128, 8, 256]
    wvtlo_v = wvtlo.rearrange("(c p) n -> p c n", p=P)

    with TileContext(nc) as tc:
        with (
            tc.tile_pool(name="const", bufs=1) as cpool,
            tc.tile_pool(name="htp", bufs=4) as htpool,
            tc.tile_pool(name="mqp", bufs=2) as mqpool,
            tc.tile_pool(name="expt", bufs=9) as epool,
            tc.tile_pool(name="ctxt", bufs=5) as ctpool,
            tc.tile_pool(name="outp", bufs=3) as opool,
            tc.tile_pool(name="recp", bufs=4) as rpool,
            tc.tile_pool(name="tmpp", bufs=5) as tpool,
            tc.tile_pool(name="psm", bufs=2, space="PSUM") as psm,
            tc.tile_pool(name="psq", bufs=2, space="PSUM") as psq,
            tc.tile_pool(name="psc", bufs=4, space="PSUM") as psc,
        ):
            ident = cpool.tile([P, P], f32, tag="ident")
            make_identity(nc, ident)

            # ht tiles: [P, d-chunk, hi/lo, s]; per-pair DMAs alternate the
            # sync and scalar issue queues (ACT is idle until phase 2) so the
            # first projection matmuls aren't gated on one sequencer.
            def load_ht(blk, eng_a, eng_b):
                t = htpool.tile([P, DC, 2, NB], f8, tag="htt")
                s0 = blk * NB
                for pi in range(NPAIR):
                    d0 = 2 * pi
                    eng = eng_a if pi % 2 == 0 else eng_b
                    eng.dma_start(
                        t[:, d0 : d0 + 2, 0, :], hthi_v[:, d0 : d0 + 2, s0 : s0 + NB]
                    )
                    eng.dma_start(
                        t[:, d0 : d0 + 2, 1, :], htlo_v[:, d0 : d0 + 2, s0 : s0 + NB]
                    )
                return t

            w_k = cpool.tile([P, DC, 2, D], f8, tag="wk")
            ht0 = htpool.tile([P, DC, 2, NB], f8, tag="htt")
            for pi in range(NPAIR):
                d0 = 2 * pi
                nc.sync.dma_start(w_k[:, d0 : d0 + 2, 0, :], wkthi_v[:, d0 : d0 + 2, :])
                nc.sync.dma_start(w_k[:, d0 : d0 + 2, 1, :], wktlo_v[:, d0 : d0 + 2, :])
                nc.scalar.dma_start(
                    ht0[:, d0 : d0 + 2, 0, :], hthi_v[:, d0 : d0 + 2, 0:NB]
                )
                nc.scalar.dma_start(
                    ht0[:, d0 : d0 + 2, 1, :], htlo_v[:, d0 : d0 + 2, 0:NB]
                )
            w_v = cpool.tile([P, DC, 2, HG * DH], f8, tag="wv")
            nc.sync.dma_start(w_v[:, :, 0, :], wvthi_v[:])
            nc.sync.dma_start(w_v[:, :, 1, :], wvtlo_v[:])
            w_q = cpool.tile([P, DC, 2, D], f8, tag="wq")
            mx = cpool.tile([P, EC * HG], f32, tag="mx")

            # kt32 = 32*k, stored as fp8 hi/lo: [P, e-chunk, hi/lo, t]
            kt = cpool.tile([P, EC, 2, S], f8, tag="kt")
            # v, padded to 128 columns: [0:64] v-head, 64 ones, [65:128] zero
            vsb = cpool.tile([P, TCH, HG, P], f16, tag="vsb")
            nc.vector.memset(vsb[:], 0.0)
            nc.vector.memset(vsb[:, :, :, DH : DH + 1], 1.0)

            def mm3(ps, lhs, rhs):
                """3-term fp8 DoubleRow accumulation over 4 chunk-pairs.
                lhs/rhs: (pair_index, hilo) -> AP with dims [P, 2, cols]."""
                for pi in range(NPAIR):
                    nc.tensor.matmul(
                        ps,
                        lhs(pi, 0),
                        rhs(pi, 0),
                        start=(pi == 0),
                        stop=False,
                        perf_mode=DR,
                    )
                    nc.tensor.matmul(
                        ps, lhs(pi, 1), rhs(pi, 0), start=False, stop=False, perf_mode=DR
                    )
                    nc.tensor.matmul(
                        ps,
                        lhs(pi, 0),
                        rhs(pi, 1),
                        start=False,
                        stop=(pi == NPAIR - 1),
                        perf_mode=DR,
                    )

            # ---- Q projection, emitted incrementally ----
            # One e-chunk "step" = 12 DoubleRow MMs into a psq tile + the
            # f16 tmp (DVE/ACT) + fp8 hi/lo (gpsimd, SBUF-only) epilogue.
            # Steps are spread through phase 1 (block 0) and the scores loop
            # (blocks 1..3) so the 3-stage epilogue chain drains at its own
            # pace without ever back-pressuring the PE through a pool.
            def make_qproj_emitter(sbi):
                htt = ht_tiles[sbi]
                mq = mqpool.tile([P, EC, 2, HG, NB], f8, tag="mq")
                state = {"e": 0}

                def emit_steps(n):
                    for _ in range(n):
                        e = state["e"]
                        if e >= EC:
                            return
                        state["e"] += 1
                        ps = psq.tile([P, NB], f32, tag="q")
                        mm3(
                            ps,
                            lambda pi, hl: w_q[:, 2 * pi : 2 * pi + 2, hl, e * P : (e + 1) * P],
                            lambda pi, hl: htt[:, 2 * pi : 2 * pi + 2, hl, :],
                        )
                        for j in range(HG):
                            col = mx[:, e * HG + j, None]
                            tmpf = tpool.tile([P, NB], f16, tag="tmpf")
                            if j % 2 == 0:
                                nc.vector.tensor_scalar(tmpf, ps, col, None, mult)
                            else:
                                nc.scalar.activation(tmpf, ps, Copy, scale=col)
                            hi = mq[:, e, 0, j, :]
                            nc.gpsimd.tensor_copy(hi, tmpf)
                            nc.gpsimd.tensor_tensor(mq[:, e, 1, j, :], tmpf, hi, sub)

                return mq, emit_steps

            # ---- phase 1: kT (all t) and v (all t) ----
            ht_tiles = {0: ht0}
            qproj0 = None
            for tb in range(SBLK):
                htt = ht_tiles.get(tb)
                if htt is None:
                    htt = load_ht(tb, nc.sync, nc.scalar)
                    ht_tiles[tb] = htt
                if tb == 1:
                    # w_q transfer starts behind ht(1); done well before the
                    # first qproj step below
                    for pi2 in range(2):
                        nc.sync.dma_start(
                            w_q[:, :, pi2, :],
                            (wqthi_v if pi2 == 0 else wqtlo_v)[:],
                        )
                    nc.sync.dma_start(mx[:], mix[:])
                    qproj0 = make_qproj_emitter(0)
                for e in range(EC):
                    ps = psm.tile([P, NB], f32, tag="m")
                    mm3(
                        ps,
                        lambda pi, hl: w_k[:, 2 * pi : 2 * pi + 2, hl, e * P : (e + 1) * P],
                        lambda pi, hl: htt[:, 2 * pi : 2 * pi + 2, hl, :],
                    )
                    # kt epilogue: hi = fp8(psum) on ACT, lo = psum - hi on DVE
                    # (gpsimd cannot access PSUM)
                    khi = kt[:, e, 0, tb * NB : (tb + 1) * NB]
                    nc.scalar.activation(khi, ps, Copy)
                    nc.vector.tensor_tensor(
                        kt[:, e, 1, tb * NB : (tb + 1) * NB], ps, khi, sub
                    )
                for ci in range(NB // P):
                    tcc = tb * (NB // P) + ci
                    ps = psm.tile([P, NB], f32, tag="m")
                    psv = ps[:, : HG * DH]
                    mm3(
                        psv,
                        lambda pi, hl: htt[:, 2 * pi : 2 * pi + 2, hl, ci * P : (ci + 1) * P],
                        lambda pi, hl: w_v[:, 2 * pi : 2 * pi + 2, hl, :],
                    )
                    for j in range(HG):
                        nc.vector.tensor_scalar(
                            vsb[:, tcc, j, 0:DH],
                            psv[:, j * DH : (j + 1) * DH],
                            1.0 / 32.0,
                            None,
                            mult,
                        )
                if tb >= 2 and qproj0 is not None:
                    qproj0[1](4)

            # ---- phase 2: per s-block ----
            def tail_mm(sbi, ctx_ps, last_exp):
                """Last ctx matmuls of block sbi + PSUM->SBUF copies."""
                for j in range(HG):
                    nc.tensor.matmul(
                        ctx_ps[j],
                        vsb[:, TCH - 1, j, :],
                        last_exp[j],
                        start=False,
                        stop=True,
                    )
                ct_tiles = []
                for j in range(HG):
                    ct = ctpool.tile([P, NB], f32, tag="ct", name=f"ct_{sbi}_{j}")
                    nc.vector.tensor_copy(ct[:], ctx_ps[j])
                    ct_tiles.append(ct)
                return ct_tiles

            def make_fin(sbi, ct_tiles):
                """Transpose + normalize + store block sbi."""

                def fin():
                    for sc in range(NB // P):
                        ob = opool.tile([P, HG * DH], f32, tag="ob", name=f"ob_{sbi}_{sc}")
                        for j in range(HG):
                            tp = psq.tile([P, P], f32, tag="q", name=f"tp_{sbi}_{sc}_{j}")
                            nc.tensor.transpose(
                                tp, ct_tiles[j][:, sc * P : (sc + 1) * P], ident
                            )
                            rc = rpool.tile([P, 1], f32, tag="rc", name=f"rc_{sbi}_{sc}_{j}")
                            nc.vector.reciprocal(rc, tp[:, DH : DH + 1])
                            nc.vector.tensor_tensor(
                                ob[:, j * DH : (j + 1) * DH],
                                tp[:, 0:DH],
                                rc[:, 0, None].to_broadcast([P, DH]),
                                mult,
                            )
                        row0 = sbi * NB + sc * P
                        # issue on SP (idle by phase 2); gpsimd's queue is
                        # deep in next-block mq hi/lo work at this point
                        nc.sync.dma_start(ctx_o[row0 : row0 + P, :], ob[:])

                return fin

            cur_emitter = qproj0
            pending_mm = None
            for sbi in range(SBLK):
                mq = cur_emitter[0]
                next_emitter = (
                    make_qproj_emitter(sbi + 1) if sbi + 1 < SBLK else None
                )

                pending_fin = None
                if pending_mm is not None:
                    prev_sbi, prev_ctx_ps, prev_last_exp = pending_mm
                    ct_tiles = tail_mm(prev_sbi, prev_ctx_ps, prev_last_exp)
                    pending_fin = make_fin(prev_sbi, ct_tiles)

                ctx_ps = [
                    psc.tile([P, NB], f32, tag="c", name=f"ctxps_{sbi}_{j}")
                    for j in range(HG)
                ]
                prev_exp = [None] * HG
                for tci in range(TCH):
                    cur_exp = []
                    for j in range(HG):
                        sp = psm.tile([P, NB], f32, tag="m")
                        mm3(
                            sp,
                            lambda pi, hl: kt[:, 2 * pi : 2 * pi + 2, hl, tci * P : (tci + 1) * P],
                            lambda pi, hl: mq[:, 2 * pi : 2 * pi + 2, hl, j, :],
                        )
                        et = epool.tile([P, NB], f16, tag="et")
                        nc.scalar.activation(et, sp, Exp, scale=EXP_SCALE)
                        cur_exp.append(et)
                    if tci > 0:
                        for j in range(HG):
                            nc.tensor.matmul(
                                ctx_ps[j],
                                vsb[:, tci - 1, j, :],
                                prev_exp[j],
                                start=(tci - 1 == 0),
                                stop=False,
                            )
                    prev_exp = cur_exp
                    if tci == 2 and pending_fin is not None:
                        pending_fin()
                        pending_fin = None
                    # one qproj step for block sbi+1 every other t-chunk:
                    # paced so the gpsimd hi/lo chain never backs up into
                    # the PE through the psq/tmp pools
                    if next_emitter is not None and tci in (1, 3, 5, 7, 9, 11, 13, 14):
                        next_emitter[1](1)

                pending_mm = (sbi, ctx_ps, prev_exp)
                cur_emitter = next_emitter

            ct_tiles = tail_mm(*pending_mm)
            make_fin(pending_mm[0], ct_tiles)()

    nc.compile()
    return nc


def make_in_maps(hidden_states, Wq, Wk, Wv, bv, mixing):
    """Host-side sharding: build per-core input dicts."""
    import ml_dtypes

    f8 = ml_dtypes.float8_e4m3
    hidden_states = np.asarray(hidden_states, dtype=np.float32)
    Wq = np.asarray(Wq, dtype=np.float32)
    Wk = np.asarray(Wk, dtype=np.float32)
    Wv = np.asarray(Wv, dtype=np.float32)
    bv = np.asarray(bv, dtype=np.float32)
    mixing = np.asarray(mixing, dtype=np.float32)

    def hilo(x):
        hi = np.ascontiguousarray(x).astype(f8)
        lo = (x - hi.astype(np.float32)).astype(f8)
        return hi, lo

    wqt_hi, wqt_lo = hilo(32.0 * Wq.T)  # [d, e]
    wkt_hi, wkt_lo = hilo(32.0 * Wk.T)

    ht_by_b = [hilo(hidden_states[b].T) for b in range(B)]

    wvT = 32.0 * Wv.T  # [d, dv]
    wvt_by_g = [hilo(wvT[:, g * HG * DH : (g + 1) * HG * DH]) for g in range(HG)]
    mix_by_g = []
    for g in range(HG):
        mrows = mixing[g * HG : (g + 1) * HG]  # [4, 1024]
        # mix[p, e*HG + j] = mixing[4g+j, e*128+p] / 4
        m = np.ascontiguousarray(
            mrows.reshape(HG, EC, P).transpose(2, 1, 0).reshape(P, EC * HG) / 4.0
        ).astype(np.float32)
        mix_by_g.append(m)

    in_maps = []
    for c in range(N_CORES):
        b, g = divmod(c, HG)
        in_maps.append(
            {
                "hthi": ht_by_b[b][0],
                "htlo": ht_by_b[b][1],
                "wqthi": wqt_hi,
                "wqtlo": wqt_lo,
                "wkthi": wkt_hi,
                "wktlo": wkt_lo,
                "wvthi": wvt_by_g[g][0],
                "wvtlo": wvt_by_g[g][1],
                "mix": mix_by_g[g],
            }
        )
    return in_maps


def assemble_output(results):
    """results: list of per-core dicts with 'ctx' [S, 256] f32. The v bias is
    added here: softmax rows sum to 1, so ctx = probs@v + bv."""
    out = np.empty((B, S, DV), dtype=np.float32)
    bv = _CACHE["bv"]
    for c in range(N_CORES):
        b, g = divmod(c, HG)
        sl = slice(g * HG * DH, (g + 1) * HG * DH)
        out[b, :, sl] = results[c]["ctx"] + bv[sl][None, :]
    return out


def _get_runner():
    """Build (once) a jitted shard_map over the 8 cores running the compiled
    Bass program via the bass_exec custom call."""
    if "runner" in _CACHE:
        return _CACHE["runner"]

    import jax
    import concourse.mybir as mybir
    from jax.sharding import Mesh, PartitionSpec
    from jax.experimental.shard_map import shard_map
    from concourse import bass2jax
    from concourse.bass2jax import _bass_exec_p, partition_id_tensor

    bass2jax.install_neuronx_cc_hook()
    nc = _CACHE.setdefault("nc", build_program())

    part_name = nc.partition_id_tensor.name if nc.partition_id_tensor else None
    dbg_name = nc.dbg_addr.name if nc.dbg_addr is not None else None
    in_names, out_names, out_avals, zero_outs = [], [], [], []
    for alloc in nc.m.functions[0].allocations:
        if not isinstance(alloc, mybir.MemoryLocationSet):
            continue
        name = alloc.memorylocations[0].name
        if alloc.kind == "ExternalInput":
            if name != part_name:
                in_names.append(name)
        elif alloc.kind == "ExternalOutput":
            out_names.append(name)
            shape = tuple(alloc.tensor_shape)
            dtype = mybir.dt.np(alloc.dtype)
            out_avals.append(jax.core.ShapedArray(shape, dtype))
            zero_outs.append(np.zeros(shape, dtype))
    n_params = len(in_names)
    all_names = in_names + out_names + ([part_name] if part_name else [])

    def _body(*args):
        operands = list(args)
        if part_name is not None:
            operands.append(partition_id_tensor())
        outs = _bass_exec_p.bind(
            *operands,
            out_avals=tuple(out_avals),
            in_names=tuple(all_names),
            out_names=tuple(out_names),
            lowering_input_output_aliases=(),
            sim_require_finite=True,
            sim_require_nnan=True,
            nc=nc,
        )
        return tuple(outs)

    devices = jax.devices()[:N_CORES]
    mesh = Mesh(np.asarray(devices), ("core",))
    spec = PartitionSpec("core")
    sharded = jax.jit(
        shard_map(
            _body,
            mesh=mesh,
            in_specs=(spec,) * (n_params + len(out_names)),
            out_specs=(spec,) * len(out_names),
            check_rep=False,
        ),
        keep_unused=True,
    )
    concat_zero = [
        np.zeros((N_CORES * z.shape[0], *z.shape[1:]), z.dtype) for z in zero_outs
    ]

    def run(in_maps):
        def core_input(c, name):
            if name == dbg_name:
                return np.zeros((1, 2), np.uint32)
            return in_maps[c][name]

        concat_in = [
            np.concatenate([core_input(c, name) for c in range(N_CORES)], axis=0)
            for name in in_names
        ]
        out_arrs = sharded(*concat_in, *concat_zero)
        return [
            {
                name: np.asarray(out_arrs[i]).reshape(
                    N_CORES, *out_avals[i].shape
                )[c]
                for i, name in enumerate(out_names)
            }
            for c in range(N_CORES)
        ]

    _CACHE["runner"] = run
    return run


def kernel(hidden_states, Wq, Wk, Wv, bv, mixing):
    run = _get_runner()
    _CACHE["bv"] = np.asarray(bv, dtype=np.float32)
    in_maps = make_in_maps(hidden_states, Wq, Wk, Wv, bv, mixing)
    return assemble_output(run(in_maps))



# revision 3
# speedup vs baseline: 1.3116x; 1.3116x over previous
"""Collaborative attention (nn_CollaborativeAttention) on 8 Trainium2 NeuronCores.

Reference math (B=2, S=2048, D=1024, H=16 heads, head mixing over full DKQ=1024):
    q = h @ Wq.T ; k = h @ Wk.T ; v = h @ Wv.T + bv
    scores[b,h,s,t] = sum_e q[b,s,e] * mixing[h,e] * k[b,t,e] / sqrt(64)
    probs = softmax_t(scores) ; ctx[b,s,:] = concat_h(probs @ v_head)

Sharding: core c handles batch b = c//4 and head group g = c%4 (4 heads each).
No cross-core communication; host slices inputs / concatenates outputs.

Matmuls run as fp8(e4m3) DoubleRow with hi/lo split-compensation:
x ~ x_hi + x_lo (both fp8, lo = fp8(x - x_hi)); projections use the three
dominant terms hi*hi + lo*hi + hi*lo.

The scores matmul uses IMPORTANCE-ORDERED mixed precision: the error
contribution of contraction index e scales with sum_j mixing[j,e]^2 (j over
this core's 4 heads), so the host permutes the e axis (columns of Wq.T/Wk.T
and the mix rows -- transparent to the math) so high-importance e's come
first. Per 256-wide chunk-pair: pairs 0-1 get all 3 terms, pair 2 drops the
kt_lo term, pair 3 keeps only hi*hi. 9 DR instructions instead of 12 at
~1.2e-2 end-to-end relative error (validated against a bit-accurate numpy
emulation of the fp8/f16 pipeline). kt_lo is only stored/produced for
chunks 0-3 and mq_lo for chunks 0-5.

The ctx accumulation runs in [s, dh] orientation: expT[t,s-sub] is the
stationary operand and v (64 cols + a ones column for the softmax
denominator) is the moving operand, so the matmul moving dim is 65 instead
of 512 with a half-zero stationary, and no PE transposes are needed: the
PSUM tile is already [s, dh] plus the denominator column, normalized by a
DVE reciprocal+multiply straight out of PSUM.

Tensors are pre-scaled so fp8 lo residuals stay above the e4m3 denormal
floor: weights are uploaded as 32*W.T, kt is stored as 32*k, mq as 8*q*mix
(mix uploaded pre-divided by 4), and exp() absorbs the 256x with scale/256.
The v bias is folded in on the host (softmax rows sum to 1 => ctx =
probs@v + bv).

Device dataflow (per core):
    kt32[e,t] hi(all)/lo(top 4 chunks) fp8 <- K projection (fp8 3-term)
    v[t,dv] f16 (+ones col)                <- V projection (fp8 3-term)
    per s-block of 512 queries (Q projection emitted one block ahead):
      mq[e,s] hi fp8 (+lo for top 6 chunks) = (32q psum) * (mix/4)
      scoresT = 9-instr fp8 DoubleRow -> exp((0.125/256)*x) -> expT[t,s] f16
      ctx_psum[s-sub, 4j, 65] += expT_chunk.T @ v_chunk (f16, moving dim 65)
      finalize: DVE reciprocal(den col) * ctx cols -> DMA out.
"""

import math

import numpy as np

B, S, D = 2, 2048, 1024
H, DV = 16, 1024
N_CORES = 8
HG = 4  # heads per core
DH = 64  # head dim
P = 128
EC = 8  # e-chunks (1024/128)
DC = 8  # d-chunks (1024/128)
NPAIR = 4  # contraction chunk-pairs (1024/256)
NB = 512  # s-block width
SBLK = 4  # number of s blocks
TCH = 16  # t-chunks of 128
KT_LO_CH = 4  # permuted e-chunks that keep the kt lo part
MQ_LO_CH = 6  # permuted e-chunks that keep the mq lo part
SCALE = 1.0 / math.sqrt(D / H)  # 0.125
EXP_SCALE = SCALE / 256.0  # psum holds (8*q*mix)*(32*k) = 256*scores

_CACHE: dict = {}


def build_program():
    """Build the (SPMD, per-core) Bass program."""
    import concourse.bass as bass
    import concourse.mybir as mybir
    from concourse import bacc
    from concourse.tile import TileContext

    f32 = mybir.dt.float32
    f16 = mybir.dt.float16
    f8 = mybir.dt.float8e4
    mult = mybir.AluOpType.mult
    sub = mybir.AluOpType.subtract
    Exp = mybir.ActivationFunctionType.Exp
    Copy = mybir.ActivationFunctionType.Copy
    DR = mybir.MatmulPerfMode.DoubleRow

    nc = bacc.Bacc("TRN2", target_bir_lowering=False, debug=True)
    hthi = nc.dram_tensor("hthi", [D, S], f8, kind="ExternalInput")
    htlo = nc.dram_tensor("htlo", [D, S], f8, kind="ExternalInput")
    wqthi = nc.dram_tensor("wqthi", [D, D], f8, kind="ExternalInput")
    wqtlo = nc.dram_tensor("wqtlo", [D, D], f8, kind="ExternalInput")
    wkthi = nc.dram_tensor("wkthi", [D, D], f8, kind="ExternalInput")
    wktlo = nc.dram_tensor("wktlo", [D, D], f8, kind="ExternalInput")
    wvthi = nc.dram_tensor("wvthi", [D, HG * DH], f8, kind="ExternalInput")
    wvtlo = nc.dram_tensor("wvtlo", [D, HG * DH], f8, kind="ExternalInput")
    mix = nc.dram_tensor("mix", [P, EC * HG], f32, kind="ExternalInput")
    ctx_o = nc.dram_tensor("ctx", [S, HG * DH], f32, kind="ExternalOutput")

    hthi_v = hthi.rearrange("(c p) s -> p c s", p=P)  # [128, 8, 2048]
    htlo_v = htlo.rearrange("(c p) s -> p c s", p=P)
    wqthi_v = wqthi.rearrange("(c p) e -> p c e", p=P)  # [128, 8, 1024]
    wqtlo_v = wqtlo.rearrange("(c p) e -> p c e", p=P)
    wkthi_v = wkthi.rearrange("(c p) e -> p c e", p=P)
    wktlo_v = wktlo.rearrange("(c p) e -> p c e", p=P)
    wvthi_v = wvthi.rearrange("(c p) n -> p c n", p=P)  # [128, 8, 256]
    wvtlo_v = wvtlo.rearrange("(c p) n -> p c n", p=P)

    with TileContext(nc) as tc:
        with (
            tc.tile_pool(name="const", bufs=1) as cpool,
            tc.tile_pool(name="htp", bufs=4) as htpool,
            tc.tile_pool(name="mqp", bufs=2) as mqpool,
            tc.tile_pool(name="expt", bufs=14) as epool,
            tc.tile_pool(name="outp", bufs=3) as opool,
            tc.tile_pool(name="recp", bufs=4) as rpool,
            tc.tile_pool(name="tmpp", bufs=5) as tpool,
            tc.tile_pool(name="psm", bufs=2, space="PSUM") as psm,
            tc.tile_pool(name="psq", bufs=2, space="PSUM") as psq,
            tc.tile_pool(name="psc", bufs=4, space="PSUM") as psc,
        ):
            # ht tiles: [P, d-chunk, hi/lo, s]; per-pair DMAs alternate the
            # sync and scalar issue queues so the first projection matmuls
            # aren't gated on one sequencer.
            def load_ht(blk, eng_a, eng_b):
                t = htpool.tile([P, DC, 2, NB], f8, tag="htt")
                s0 = blk * NB
                for pi in range(NPAIR):
                    d0 = 2 * pi
                    eng = eng_a if pi % 2 == 0 else eng_b
                    eng.dma_start(
                        t[:, d0 : d0 + 2, 0, :], hthi_v[:, d0 : d0 + 2, s0 : s0 + NB]
                    )
                    eng.dma_start(
                        t[:, d0 : d0 + 2, 1, :], htlo_v[:, d0 : d0 + 2, s0 : s0 + NB]
                    )
                return t

            w_k = cpool.tile([P, DC, 2, D], f8, tag="wk")
            ht0 = htpool.tile([P, DC, 2, NB], f8, tag="htt")
            for pi in range(NPAIR):
                d0 = 2 * pi
                nc.sync.dma_start(w_k[:, d0 : d0 + 2, 0, :], wkthi_v[:, d0 : d0 + 2, :])
                nc.sync.dma_start(w_k[:, d0 : d0 + 2, 1, :], wktlo_v[:, d0 : d0 + 2, :])
                nc.scalar.dma_start(
                    ht0[:, d0 : d0 + 2, 0, :], hthi_v[:, d0 : d0 + 2, 0:NB]
                )
                nc.scalar.dma_start(
                    ht0[:, d0 : d0 + 2, 1, :], htlo_v[:, d0 : d0 + 2, 0:NB]
                )
            w_v = cpool.tile([P, DC, 2, HG * DH], f8, tag="wv")
            nc.sync.dma_start(w_v[:, :, 0, :], wvthi_v[:])
            nc.sync.dma_start(w_v[:, :, 1, :], wvtlo_v[:])
            w_q = cpool.tile([P, DC, 2, D], f8, tag="wq")
            mx = cpool.tile([P, EC * HG], f32, tag="mx")

            # kt32 = 32*k, fp8: hi for all chunks, lo only for top KT_LO_CH
            kt_h = cpool.tile([P, EC, S], f8, tag="kth")
            kt_l = cpool.tile([P, KT_LO_CH, S], f8, tag="ktl")
            # v, 64 head cols + ones col 64 (softmax denominator)
            vsb = cpool.tile([P, TCH, HG, DH + 1], f16, tag="vsb")
            nc.vector.memset(vsb[:, :, :, DH : DH + 1], 1.0)

            def mm3(ps, lhs, rhs):
                """3-term fp8 DoubleRow accumulation over 4 chunk-pairs.
                lhs/rhs: (pair_index, hilo) -> AP with dims [P, 2, cols]."""
                for pi in range(NPAIR):
                    nc.tensor.matmul(
                        ps,
                        lhs(pi, 0),
                        rhs(pi, 0),
                        start=(pi == 0),
                        stop=False,
                        perf_mode=DR,
                    )
                    nc.tensor.matmul(
                        ps, lhs(pi, 1), rhs(pi, 0), start=False, stop=False, perf_mode=DR
                    )
                    nc.tensor.matmul(
                        ps,
                        lhs(pi, 0),
                        rhs(pi, 1),
                        start=False,
                        stop=(pi == NPAIR - 1),
                        perf_mode=DR,
                    )

            # ---- Q projection, emitted incrementally ----
            # One e-chunk "step" = 12 DoubleRow MMs into a psq tile + the mq
            # epilogue: top MQ_LO_CH chunks get the f16 tmp (DVE/ACT) + fp8
            # hi/lo (gpsimd) path; the rest write fp8 hi directly from PSUM.
            def make_qproj_emitter(sbi):
                htt = ht_tiles[sbi]
                mqh = mqpool.tile([P, EC, HG, NB], f8, tag="mqh")
                mql = mqpool.tile([P, MQ_LO_CH, HG, NB], f8, tag="mql")
                state = {"e": 0}

                def emit_steps(n):
                    for _ in range(n):
                        e = state["e"]
                        if e >= EC:
                            return
                        state["e"] += 1
                        ps = psq.tile([P, NB], f32, tag="q")
                        mm3(
                            ps,
                            lambda pi, hl: w_q[:, 2 * pi : 2 * pi + 2, hl, e * P : (e + 1) * P],
                            lambda pi, hl: htt[:, 2 * pi : 2 * pi + 2, hl, :],
                        )
                        for j in range(HG):
                            col = mx[:, e * HG + j, None]
                            if e < MQ_LO_CH:
                                tmpf = tpool.tile([P, NB], f16, tag="tmpf")
                                if j % 2 == 0:
                                    nc.vector.tensor_scalar(tmpf, ps, col, None, mult)
                                else:
                                    nc.scalar.activation(tmpf, ps, Copy, scale=col)
                                hi = mqh[:, e, j, :]
                                nc.gpsimd.tensor_copy(hi, tmpf)
                                nc.gpsimd.tensor_tensor(
                                    mql[:, e, j, :], tmpf, hi, sub
                                )
                            else:
                                if j % 2 == 0:
                                    nc.vector.tensor_scalar(
                                        mqh[:, e, j, :], ps, col, None, mult
                                    )
                                else:
                                    nc.scalar.activation(
                                        mqh[:, e, j, :], ps, Copy, scale=col
                                    )

                return (mqh, mql), emit_steps

            # ---- phase 1: kT (all t) and v (all t) ----
            ht_tiles = {0: ht0}
            qproj0 = None
            for tb in range(SBLK):
                htt = ht_tiles.get(tb)
                if htt is None:
                    htt = load_ht(tb, nc.sync, nc.scalar)
                    ht_tiles[tb] = htt
                if tb == 1:
                    for pi2 in range(2):
                        nc.sync.dma_start(
                            w_q[:, :, pi2, :],
                            (wqthi_v if pi2 == 0 else wqtlo_v)[:],
                        )
                    nc.sync.dma_start(mx[:], mix[:])
                    qproj0 = make_qproj_emitter(0)
                for e in range(EC):
                    ps = psm.tile([P, NB], f32, tag="m")
                    mm3(
                        ps,
                        lambda pi, hl: w_k[:, 2 * pi : 2 * pi + 2, hl, e * P : (e + 1) * P],
                        lambda pi, hl: htt[:, 2 * pi : 2 * pi + 2, hl, :],
                    )
                    # kt epilogue: hi = fp8(psum) on ACT, lo = psum - hi on DVE
                    khi = kt_h[:, e, tb * NB : (tb + 1) * NB]
                    nc.scalar.activation(khi, ps, Copy)
                    if e < KT_LO_CH:
                        nc.vector.tensor_tensor(
                            kt_l[:, e, tb * NB : (tb + 1) * NB], ps, khi, sub
                        )
                for ci in range(NB // P):
                    tcc = tb * (NB // P) + ci
                    ps = psm.tile([P, NB], f32, tag="m")
                    psv = ps[:, : HG * DH]
                    mm3(
                        psv,
                        lambda pi, hl: htt[:, 2 * pi : 2 * pi + 2, hl, ci * P : (ci + 1) * P],
                        lambda pi, hl: w_v[:, 2 * pi : 2 * pi + 2, hl, :],
                    )
                    for j in range(HG):
                        nc.vector.tensor_scalar(
                            vsb[:, tcc, j, 0:DH],
                            psv[:, j * DH : (j + 1) * DH],
                            1.0 / 32.0,
                            None,
                            mult,
                        )
                if tb >= 2 and qproj0 is not None:
                    qproj0[1](4)

            # ---- phase 2: per s-block ----
            def score_mm(sp, mqh, mql, j, tci):
                """Importance-pruned fp8 DoubleRow scores: 9 instructions."""
                t0 = tci * P
                plan = []
                for p in range(NPAIR):
                    c0 = 2 * p
                    plan.append((c0, 0, 0))
                    if c0 < KT_LO_CH:
                        plan.append((c0, 1, 0))
                    if c0 < MQ_LO_CH:
                        plan.append((c0, 0, 1))
                for i, (c0, ks, ms) in enumerate(plan):
                    kop = (
                        kt_h[:, c0 : c0 + 2, t0 : t0 + P]
                        if ks == 0
                        else kt_l[:, c0 : c0 + 2, t0 : t0 + P]
                    )
                    mop = (
                        mqh[:, c0 : c0 + 2, j, :]
                        if ms == 0
                        else mql[:, c0 : c0 + 2, j, :]
                    )
                    nc.tensor.matmul(
                        sp,
                        kop,
                        mop,
                        start=(i == 0),
                        stop=(i == len(plan) - 1),
                        perf_mode=DR,
                    )

            def emit_ctx(ctxa, tc_i, ets):
                """ctx accumulation for t-chunk tc_i: [s-sub, dh+1] psum.

                One PSUM accumulation group per ctxa bank: start on the first
                matmul into the tile (zeroes the whole 2KB region; later
                first-writes to pending-zero bytes overwrite), stop on the
                last."""
                for j in range(HG):
                    for sc in range(NB // P):
                        nc.tensor.matmul(
                            ctxa[sc][:, j, :],
                            ets[j][:, sc * P : (sc + 1) * P],
                            vsb[:, tc_i, j, :],
                            start=(tc_i == 0 and j == 0),
                            stop=(tc_i == TCH - 1 and j == HG - 1),
                        )

            def finalize(sbi, ctxa):
                """Normalize by the denominator column and store block sbi."""
                for sc in range(NB // P):
                    ob = opool.tile([P, HG * DH], f32, tag="ob", name=f"ob_{sbi}_{sc}")
                    for j in range(HG):
                        rc = rpool.tile([P, 1], f32, tag="rc", name=f"rc_{sbi}_{sc}_{j}")
                        nc.vector.reciprocal(rc, ctxa[sc][:, j, DH : DH + 1])
                        nc.vector.tensor_tensor(
                            ob[:, j * DH : (j + 1) * DH],
                            ctxa[sc][:, j, 0:DH],
                            rc[:, 0, None].to_broadcast([P, DH]),
                            mult,
                        )
                    row0 = sbi * NB + sc * P
                    nc.sync.dma_start(ctx_o[row0 : row0 + P, :], ob[:])

            cur_emitter = qproj0
            pending = None  # (sbi, ctxa, {tci: ets}) for last two t-chunks
            for sbi in range(SBLK):
                mqh, mql = cur_emitter[0]
                next_emitter = (
                    make_qproj_emitter(sbi + 1) if sbi + 1 < SBLK else None
                )

                # finish the previous block: tail ctx matmuls + finalize
                if pending is not None:
                    p_sbi, p_ctxa, p_tail = pending
                    for tc_i in sorted(p_tail):
                        emit_ctx(p_ctxa, tc_i, p_tail[tc_i])
                    finalize(p_sbi, p_ctxa)
                    pending = None

                ctxa = [
                    psc.tile([P, HG, DH + 1], f32, tag="c", name=f"ctxa_{sbi}_{sc}")
                    for sc in range(NB // P)
                ]
                live_exp = {}
                for tci in range(TCH):
                    cur = []
                    for j in range(HG):
                        sp = psm.tile([P, NB], f32, tag="m")
                        score_mm(sp, mqh, mql, j, tci)
                        et = epool.tile([P, NB], f16, tag="et")
                        nc.scalar.activation(et, sp, Exp, scale=EXP_SCALE)
                        cur.append(et)
                    live_exp[tci] = cur
                    # ctx for tci-2: leaves slack for finalize of the
                    # previous block to release the psc banks
                    if tci >= 2:
                        emit_ctx(ctxa, tci - 2, live_exp.pop(tci - 2))
                    # one qproj step for block sbi+1 every other t-chunk
                    if next_emitter is not None and tci in (1, 3, 5, 7, 9, 11, 13, 14):
                        next_emitter[1](1)

                pending = (sbi, ctxa, live_exp)
                cur_emitter = next_emitter

            p_sbi, p_ctxa, p_tail = pending
            for tc_i in sorted(p_tail):
                emit_ctx(p_ctxa, tc_i, p_tail[tc_i])
            finalize(p_sbi, p_ctxa)

    nc.compile()
    return nc


def make_in_maps(hidden_states, Wq, Wk, Wv, bv, mixing):
    """Host-side sharding: build per-core input dicts."""
    import ml_dtypes

    f8 = ml_dtypes.float8_e4m3
    hidden_states = np.asarray(hidden_states, dtype=np.float32)
    Wq = np.asarray(Wq, dtype=np.float32)
    Wk = np.asarray(Wk, dtype=np.float32)
    Wv = np.asarray(Wv, dtype=np.float32)
    bv = np.asarray(bv, dtype=np.float32)
    mixing = np.asarray(mixing, dtype=np.float32)

    def hilo(x):
        hi = np.ascontiguousarray(x).astype(f8)
        lo = (x - hi.astype(np.float32)).astype(f8)
        return hi, lo

    wqT = 32.0 * Wq.T  # [d, e]
    wkT = 32.0 * Wk.T
    ht_by_b = [hilo(hidden_states[b].T) for b in range(B)]

    wvT = 32.0 * Wv.T  # [d, dv]
    wvt_by_g = [hilo(wvT[:, g * HG * DH : (g + 1) * HG * DH]) for g in range(HG)]

    # per-group importance permutation of the e axis: sort by
    # sum_j mixing[j,e]^2 descending so low-importance e's land in the
    # term-pruned chunks.
    wq_by_g, wk_by_g, mix_by_g = [], [], []
    for g in range(HG):
        mrows = mixing[g * HG : (g + 1) * HG]  # [4, 1024]
        imp = (mrows**2).sum(axis=0)
        perm = np.argsort(-imp)
        wq_by_g.append(hilo(wqT[:, perm]))
        wk_by_g.append(hilo(wkT[:, perm]))
        mperm = mrows[:, perm]  # [4, 1024]
        # mix[p, e*HG + j] = mperm[j, e*128+p] / 4
        m = np.ascontiguousarray(
            mperm.reshape(HG, EC, P).transpose(2, 1, 0).reshape(P, EC * HG) / 4.0
        ).astype(np.float32)
        mix_by_g.append(m)

    in_maps = []
    for c in range(N_CORES):
        b, g = divmod(c, HG)
        in_maps.append(
            {
                "hthi": ht_by_b[b][0],
                "htlo": ht_by_b[b][1],
                "wqthi": wq_by_g[g][0],
                "wqtlo": wq_by_g[g][1],
                "wkthi": wk_by_g[g][0],
                "wktlo": wk_by_g[g][1],
                "wvthi": wvt_by_g[g][0],
                "wvtlo": wvt_by_g[g][1],
                "mix": mix_by_g[g],
            }
        )
    return in_maps


def assemble_output(results):
    """results: list of per-core dicts with 'ctx' [S, 256] f32. The v bias is
    added here: softmax rows sum to 1, so ctx = probs@v + bv."""
    out = np.empty((B, S, DV), dtype=np.float32)
    bv = _CACHE["bv"]
    for c in range(N_CORES):
        b, g = divmod(c, HG)
        sl = slice(g * HG * DH, (g + 1) * HG * DH)
        out[b, :, sl] = results[c]["ctx"] + bv[sl][None, :]
    return out


def _get_runner():
    """Build (once) a jitted shard_map over the 8 cores running the compiled
    Bass program via the bass_exec custom call."""
    if "runner" in _CACHE:
        return _CACHE["runner"]

    import jax
    import concourse.mybir as mybir
    from jax.sharding import Mesh, PartitionSpec
    from jax.experimental.shard_map import shard_map
    from concourse import bass2jax
    from concourse.bass2jax import _bass_exec_p, partition_id_tensor

    bass2jax.install_neuronx_cc_hook()
    nc = _CACHE.setdefault("nc", build_program())

    part_name = nc.partition_id_tensor.name if nc.partition_id_tensor else None
    dbg_name = nc.dbg_addr.name if nc.dbg_addr is not None else None
    in_names, out_names, out_avals, zero_outs = [], [], [], []
    for alloc in nc.m.functions[0].allocations:
        if not isinstance(alloc, mybir.MemoryLocationSet):
            continue
        name = alloc.memorylocations[0].name
        if alloc.kind == "ExternalInput":
            if name != part_name:
                in_names.append(name)
        elif alloc.kind == "ExternalOutput":
            out_names.append(name)
            shape = tuple(alloc.tensor_shape)
            dtype = mybir.dt.np(alloc.dtype)
            out_avals.append(jax.core.ShapedArray(shape, dtype))
            zero_outs.append(np.zeros(shape, dtype))
    n_params = len(in_names)
    all_names = in_names + out_names + ([part_name] if part_name else [])

    def _body(*args):
        operands = list(args)
        if part_name is not None:
            operands.append(partition_id_tensor())
        outs = _bass_exec_p.bind(
            *operands,
            out_avals=tuple(out_avals),
            in_names=tuple(all_names),
            out_names=tuple(out_names),
            lowering_input_output_aliases=(),
            sim_require_finite=True,
            sim_require_nnan=True,
            nc=nc,
        )
        return tuple(outs)

    devices = jax.devices()[:N_CORES]
    mesh = Mesh(np.asarray(devices), ("core",))
    spec = PartitionSpec("core")
    sharded = jax.jit(
        shard_map(
            _body,
            mesh=mesh,
            in_specs=(spec,) * (n_params + len(out_names)),
            out_specs=(spec,) * len(out_names),
            check_rep=False,
        ),
        keep_unused=True,
    )
    concat_zero = [
        np.zeros((N_CORES * z.shape[0], *z.shape[1:]), z.dtype) for z in zero_outs
    ]

    def run(in_maps):
        def core_input(c, name):
            if name == dbg_name:
                return np.zeros((1, 2), np.uint32)
            return in_maps[c][name]

        concat_in = [
            np.concatenate([core_input(c, name) for c in range(N_CORES)], axis=0)
            for name in in_names
        ]
        out_arrs = sharded(*concat_in, *concat_zero)
        return [
            {
                name: np.asarray(out_arrs[i]).reshape(
                    N_CORES, *out_avals[i].shape
                )[c]
                for i, name in enumerate(out_names)
            }
            for c in range(N_CORES)
        ]

    _CACHE["runner"] = run
    return run


def kernel(hidden_states, Wq, Wk, Wv, bv, mixing):
    run = _get_runner()
    _CACHE["bv"] = np.asarray(bv, dtype=np.float32)
    in_maps = make_in_maps(hidden_states, Wq, Wk, Wv, bv, mixing)
    return assemble_output(run(in_maps))


# revision 15
# speedup vs baseline: 1.3713x; 1.0455x over previous
"""Collaborative attention (nn_CollaborativeAttention) on 8 Trainium2 NeuronCores.

Reference math (B=2, S=2048, D=1024, H=16 heads, head mixing over full DKQ=1024):
    q = h @ Wq.T ; k = h @ Wk.T ; v = h @ Wv.T + bv
    scores[b,h,s,t] = sum_e q[b,s,e] * mixing[h,e] * k[b,t,e] / sqrt(64)
    probs = softmax_t(scores) ; ctx[b,s,:] = concat_h(probs @ v_head)

Sharding: core c handles batch b = c//4 and head group g = c%4 (4 heads each).
No cross-core communication; host slices inputs / concatenates outputs.

Matmuls run as fp8(e4m3) DoubleRow with hi/lo split-compensation:
x ~ x_hi + x_lo (both fp8, lo = fp8(x - x_hi)); projections use the three
dominant terms hi*hi + lo*hi + hi*lo.

The scores matmul uses IMPORTANCE-ORDERED mixed precision: the error
contribution of contraction index e scales with sum_j mixing[j,e]^2 (j over
this core's 4 heads), so the host permutes the e axis (columns of Wq.T/Wk.T
and the mix rows -- transparent to the math) so high-importance e's come
first. Per 256-wide chunk-pair: pairs 0-1 get all 3 terms, pair 2 drops the
kt_lo term, pair 3 keeps only hi*hi. 9 DR instructions instead of 12 at
~1.2e-2 end-to-end relative error (validated against a bit-accurate numpy
emulation of the fp8/f16 pipeline). kt_lo is only stored/produced for
chunks 0-3 and mq_lo for chunks 0-5.

The ctx accumulation runs in [s, dh] orientation: expT[t,s-sub] is the
stationary operand and v (64 cols + a ones column for the softmax
denominator) is the moving operand, so the matmul moving dim is 65 instead
of 512 with a half-zero stationary, and no PE transposes are needed: the
PSUM tile is already [s, dh] plus the denominator column, normalized by a
DVE reciprocal+multiply straight out of PSUM.

Tensors are pre-scaled so fp8 lo residuals stay above the e4m3 denormal
floor: weights are uploaded as 32*W.T, kt is stored as 32*k, mq as 8*q*mix
(mix uploaded pre-divided by 4), and exp() absorbs the 256x with scale/256.
The v bias is folded in on the host (softmax rows sum to 1 => ctx =
probs@v + bv).

Device dataflow (per core):
    kt32[e,t] hi(all)/lo(top 4 chunks) fp8 <- K projection (fp8 3-term)
    v[t,dv] f16 (+ones col)                <- V projection (fp8 3-term)
    per s-block of 512 queries (Q projection emitted one block ahead):
      mq[e,s] hi fp8 (+lo for top 6 chunks) = (32q psum) * (mix/4)
      scoresT = 9-instr fp8 DoubleRow -> exp((0.125/256)*x) -> expT[t,s] f16
      ctx_psum[s-sub, 4j, 65] += expT_chunk.T @ v_chunk (f16, moving dim 65)
      finalize: DVE reciprocal(den col) * ctx cols -> DMA out.
"""

import math

import numpy as np

B, S, D = 2, 2048, 1024
H, DV = 16, 1024
N_CORES = 8
HG = 4  # heads per core
DH = 64  # head dim
P = 128
EC = 8  # e-chunks (1024/128)
DC = 8  # d-chunks (1024/128)
NPAIR = 4  # contraction chunk-pairs (1024/256)
NB = 512  # s-block width
SBLK = 4  # number of s blocks
TCH = 16  # t-chunks of 128
KT_LO_CH = 4  # permuted e-chunks that keep the kt lo part
MQ_LO_CH = 6  # permuted e-chunks that keep the mq lo part
SCALE = 1.0 / math.sqrt(D / H)  # 0.125
EXP_SCALE = SCALE / 256.0  # psum holds (8*q*mix)*(32*k) = 256*scores

_CACHE: dict = {}


def build_program():
    """Build the (SPMD, per-core) Bass program."""
    import concourse.bass as bass
    import concourse.mybir as mybir
    from concourse import bacc
    from concourse.tile import TileContext

    f32 = mybir.dt.float32
    f16 = mybir.dt.float16
    f8 = mybir.dt.float8e4
    mult = mybir.AluOpType.mult
    sub = mybir.AluOpType.subtract
    Exp = mybir.ActivationFunctionType.Exp
    Copy = mybir.ActivationFunctionType.Copy
    DR = mybir.MatmulPerfMode.DoubleRow

    nc = bacc.Bacc("TRN2", target_bir_lowering=False, debug=True)
    hthi = nc.dram_tensor("hthi", [D, S], f8, kind="ExternalInput")
    htlo = nc.dram_tensor("htlo", [D, S], f8, kind="ExternalInput")
    wqthi = nc.dram_tensor("wqthi", [D, D], f8, kind="ExternalInput")
    wqtlo = nc.dram_tensor("wqtlo", [D, D], f8, kind="ExternalInput")
    wkthi = nc.dram_tensor("wkthi", [D, D], f8, kind="ExternalInput")
    wktlo = nc.dram_tensor("wktlo", [D, D], f8, kind="ExternalInput")
    wvthi = nc.dram_tensor("wvthi", [D, HG * DH], f8, kind="ExternalInput")
    wvtlo = nc.dram_tensor("wvtlo", [D, HG * DH], f8, kind="ExternalInput")
    mix = nc.dram_tensor("mix", [P, EC * HG], f32, kind="ExternalInput")
    ctx_o = nc.dram_tensor("ctx", [S, HG * DH], f32, kind="ExternalOutput")

    hthi_v = hthi.rearrange("(c p) s -> p c s", p=P)  # [128, 8, 2048]
    htlo_v = htlo.rearrange("(c p) s -> p c s", p=P)
    wqthi_v = wqthi.rearrange("(c p) e -> p c e", p=P)  # [128, 8, 1024]
    wqtlo_v = wqtlo.rearrange("(c p) e -> p c e", p=P)
    wkthi_v = wkthi.rearrange("(c p) e -> p c e", p=P)
    wktlo_v = wktlo.rearrange("(c p) e -> p c e", p=P)
    wvthi_v = wvthi.rearrange("(c p) n -> p c n", p=P)  # [128, 8, 256]
    wvtlo_v = wvtlo.rearrange("(c p) n -> p c n", p=P)

    with TileContext(nc) as tc:
        with (
            tc.tile_pool(name="const", bufs=1) as cpool,
            tc.tile_pool(name="htp", bufs=4) as htpool,
            tc.tile_pool(name="mqp", bufs=2) as mqpool,
            tc.tile_pool(name="expt", bufs=14) as epool,
            tc.tile_pool(name="outp", bufs=4) as opool,
            tc.tile_pool(name="recp", bufs=4) as rpool,
            tc.tile_pool(name="tmpp", bufs=5) as tpool,
            tc.tile_pool(name="psm", bufs=2, space="PSUM") as psm,
            tc.tile_pool(name="psq", bufs=2, space="PSUM") as psq,
            tc.tile_pool(name="psc", bufs=4, space="PSUM") as psc,
        ):
            # ht tiles: [P, d-chunk, hi/lo, s]; per-pair DMAs alternate the
            # sync and scalar issue queues so the first projection matmuls
            # aren't gated on one sequencer.
            def load_ht(blk, eng_a, eng_b):
                t = htpool.tile([P, DC, 2, NB], f8, tag="htt")
                s0 = blk * NB
                for pi in range(NPAIR):
                    d0 = 2 * pi
                    eng = eng_a if pi % 2 == 0 else eng_b
                    eng.dma_start(
                        t[:, d0 : d0 + 2, 0, :], hthi_v[:, d0 : d0 + 2, s0 : s0 + NB]
                    )
                    eng.dma_start(
                        t[:, d0 : d0 + 2, 1, :], htlo_v[:, d0 : d0 + 2, s0 : s0 + NB]
                    )
                return t

            w_k = cpool.tile([P, DC, 2, D], f8, tag="wk")
            w_v = cpool.tile([P, DC, 2, HG * DH], f8, tag="wv")
            ht0 = htpool.tile([P, DC, 2, NB], f8, tag="htt")
            # hi parts ahead of lo parts and w_v (0.5MB) ahead of w_k (2MB) on
            # the shared DMA device: the V projection of block 0 and the
            # term-major hi*hi matmuls start before the lo halves land.
            nc.sync.dma_start(w_v[:, :, 0, :], wvthi_v[:])
            for pi in range(NPAIR):
                d0 = 2 * pi
                nc.scalar.dma_start(
                    ht0[:, d0 : d0 + 2, 0, :], hthi_v[:, d0 : d0 + 2, 0:NB]
                )
            nc.sync.dma_start(w_v[:, :, 1, :], wvtlo_v[:])
            for pi in range(NPAIR):
                d0 = 2 * pi
                nc.scalar.dma_start(
                    ht0[:, d0 : d0 + 2, 1, :], htlo_v[:, d0 : d0 + 2, 0:NB]
                )
            for pi in range(NPAIR):
                d0 = 2 * pi
                nc.sync.dma_start(w_k[:, d0 : d0 + 2, 0, :], wkthi_v[:, d0 : d0 + 2, :])
            for pi in range(NPAIR):
                d0 = 2 * pi
                nc.sync.dma_start(w_k[:, d0 : d0 + 2, 1, :], wktlo_v[:, d0 : d0 + 2, :])
            w_q = cpool.tile([P, DC, 2, D], f8, tag="wq")
            mx = cpool.tile([P, EC * HG], f32, tag="mx")

            # kt32 = 32*k, fp8: hi for all chunks, lo only for top KT_LO_CH
            kt_h = cpool.tile([P, EC, S], f8, tag="kth")
            kt_l = cpool.tile([P, KT_LO_CH, S], f8, tag="ktl")
            # v, 64 head cols + ones col 64 (softmax denominator)
            vsb = cpool.tile([P, TCH, HG, DH + 1], f16, tag="vsb")
            nc.vector.memset(vsb[:, :, :, DH : DH + 1], 1.0)

            def mm3(ps, lhs, rhs):
                """3-term fp8 DoubleRow accumulation over 4 chunk-pairs.
                lhs/rhs: (pair_index, hilo) -> AP with dims [P, 2, cols].
                Term-major order (all hi*hi first) so the first instructions
                only gate on the hi-part DMAs at kernel start."""
                terms = [(0, 0), (1, 0), (0, 1)]
                n = 0
                for (lh, rh) in terms:
                    for pi in range(NPAIR):
                        nc.tensor.matmul(
                            ps,
                            lhs(pi, lh),
                            rhs(pi, rh),
                            start=(n == 0),
                            stop=(n == 3 * NPAIR - 1),
                            perf_mode=DR,
                        )
                        n += 1

            # ---- Q projection, emitted incrementally ----
            # One e-chunk "step" = 12 DoubleRow MMs into a psq tile + the mq
            # epilogue: top MQ_LO_CH chunks get the f16 tmp (DVE/ACT) + fp8
            # hi/lo (gpsimd) path; the rest write fp8 hi directly from PSUM.
            def make_qproj_emitter(sbi):
                htt = ht_tiles[sbi]
                mqh = mqpool.tile([P, EC, HG, NB], f8, tag="mqh")
                mql = mqpool.tile([P, MQ_LO_CH, HG, NB], f8, tag="mql")
                state = {"e": 0}

                def emit_steps(n, all_dve=False):
                    for _ in range(n):
                        e = state["e"]
                        if e >= EC:
                            return
                        state["e"] += 1
                        ps = psq.tile([P, NB], f32, tag="q")
                        mm3(
                            ps,
                            lambda pi, hl: w_q[:, 2 * pi : 2 * pi + 2, hl, e * P : (e + 1) * P],
                            lambda pi, hl: htt[:, 2 * pi : 2 * pi + 2, hl, :],
                        )
                        for j in range(HG):
                            col = mx[:, e * HG + j, None]
                            # in phase 1 ACT is busy with the kt-hi epilogue,
                            # so route all psum reads to DVE there
                            use_dve = all_dve or j % 2 == 0
                            if e < MQ_LO_CH:
                                tmpf = tpool.tile([P, NB], f16, tag="tmpf")
                                if use_dve:
                                    nc.vector.tensor_scalar(tmpf, ps, col, None, mult)
                                else:
                                    nc.scalar.activation(tmpf, ps, Copy, scale=col)
                                hi = mqh[:, e, j, :]
                                nc.gpsimd.tensor_copy(hi, tmpf)
                                nc.gpsimd.tensor_tensor(
                                    mql[:, e, j, :], tmpf, hi, sub
                                )
                            else:
                                if use_dve:
                                    nc.vector.tensor_scalar(
                                        mqh[:, e, j, :], ps, col, None, mult
                                    )
                                else:
                                    nc.scalar.activation(
                                        mqh[:, e, j, :], ps, Copy, scale=col
                                    )

                return (mqh, mql), emit_steps

            # ---- phase 1: kT (all t) and v (all t) ----
            ht_tiles = {0: ht0}
            qproj0 = None

            def v_proj(tb, htt):
                for ci in range(NB // P):
                    tcc = tb * (NB // P) + ci
                    ps = psm.tile([P, NB], f32, tag="m")
                    psv = ps[:, : HG * DH]
                    mm3(
                        psv,
                        lambda pi, hl: htt[:, 2 * pi : 2 * pi + 2, hl, ci * P : (ci + 1) * P],
                        lambda pi, hl: w_v[:, 2 * pi : 2 * pi + 2, hl, :],
                    )
                    nc.vector.tensor_scalar(
                        vsb[:, tcc, :, 0:DH], psv, 1.0 / 32.0, None, mult
                    )

            for tb in range(SBLK):
                htt = ht_tiles.get(tb)
                if htt is None:
                    htt = load_ht(tb, nc.sync, nc.scalar)
                    ht_tiles[tb] = htt
                if tb == 1:
                    # w_q/mix on the gpsimd issue queue (sync carries the ht
                    # block loads; gpsimd is idle until the qproj epilogues)
                    for pi2 in range(2):
                        nc.gpsimd.dma_start(
                            w_q[:, :, pi2, :],
                            (wqthi_v if pi2 == 0 else wqtlo_v)[:],
                        )
                    nc.gpsimd.dma_start(mx[:], mix[:])
                    qproj0 = make_qproj_emitter(0)
                # block 0: V first -- w_v lands well before w_k
                if tb == 0:
                    v_proj(tb, htt)
                for e in range(EC):
                    ps = psm.tile([P, NB], f32, tag="m")
                    mm3(
                        ps,
                        lambda pi, hl: w_k[:, 2 * pi : 2 * pi + 2, hl, e * P : (e + 1) * P],
                        lambda pi, hl: htt[:, 2 * pi : 2 * pi + 2, hl, :],
                    )
                    # kt epilogue: hi = fp8(psum) on ACT, lo = psum - hi on DVE
                    khi = kt_h[:, e, tb * NB : (tb + 1) * NB]
                    nc.scalar.activation(khi, ps, Copy)
                    if e < KT_LO_CH:
                        nc.vector.tensor_tensor(
                            kt_l[:, e, tb * NB : (tb + 1) * NB], ps, khi, sub
                        )
                if tb > 0:
                    v_proj(tb, htt)
                if tb >= 2 and qproj0 is not None:
                    qproj0[1](4, all_dve=True)

            # ---- phase 2: per s-block ----
            def score_mm(sp, mqh, mql, j, tci):
                """Importance-pruned fp8 DoubleRow scores: 8.5 instructions
                on average -- the kt_lo term of pair 1 is applied on even
                t-chunks only (its variance contribution is down-weighted by
                the importance permutation, and halving it costs ~sqrt(2)
                less error than dropping it)."""
                t0 = tci * P
                plan = []
                for p in range(NPAIR):
                    c0 = 2 * p
                    plan.append((c0, 0, 0))
                    if c0 == 0 or (c0 == 2 and tci % 2 == 0):
                        plan.append((c0, 1, 0))
                    if c0 < MQ_LO_CH:
                        plan.append((c0, 0, 1))
                for i, (c0, ks, ms) in enumerate(plan):
                    kop = (
                        kt_h[:, c0 : c0 + 2, t0 : t0 + P]
                        if ks == 0
                        else kt_l[:, c0 : c0 + 2, t0 : t0 + P]
                    )
                    mop = (
                        mqh[:, c0 : c0 + 2, j, :]
                        if ms == 0
                        else mql[:, c0 : c0 + 2, j, :]
                    )
                    nc.tensor.matmul(
                        sp,
                        kop,
                        mop,
                        start=(i == 0),
                        stop=(i == len(plan) - 1),
                        perf_mode=DR,
                    )

            def emit_ctx(ctxa, tc_i, ets, sc_list=None):
                """ctx accumulation for t-chunk tc_i: [s-sub, dh+1] psum.

                One PSUM accumulation group per ctxa bank: start on the first
                matmul into the tile (zeroes the whole 2KB region; later
                first-writes to pending-zero bytes overwrite), stop on the
                last."""
                for sc in sc_list if sc_list is not None else range(NB // P):
                    for j in range(HG):
                        nc.tensor.matmul(
                            ctxa[sc][:, j, :],
                            ets[j][:, sc * P : (sc + 1) * P],
                            vsb[:, tc_i, j, :],
                            start=(tc_i == 0 and j == 0),
                            stop=(tc_i == TCH - 1 and j == HG - 1),
                        )

            def finalize_sc(sbi, ctxa, sc):
                """Normalize one 128-row s-chunk by its denominator column.
                One batched reciprocal; the multiplies alternate DVE/ACT so
                the end-of-kernel chain is half as deep on either engine."""
                ob = opool.tile([P, HG * DH], f32, tag="ob", name=f"ob_{sbi}_{sc}")
                rc = rpool.tile([P, HG], f32, tag="rc", name=f"rc_{sbi}_{sc}")
                nc.vector.reciprocal(rc, ctxa[sc][:, :, DH : DH + 1])
                for j in range(HG):
                    if j % 2 == 0:
                        nc.vector.tensor_tensor(
                            ob[:, j * DH : (j + 1) * DH],
                            ctxa[sc][:, j, 0:DH],
                            rc[:, j, None].to_broadcast([P, DH]),
                            mult,
                        )
                    else:
                        nc.scalar.activation(
                            ob[:, j * DH : (j + 1) * DH],
                            ctxa[sc][:, j, 0:DH],
                            Copy,
                            scale=rc[:, j, None],
                        )
                row0 = sbi * NB + sc * P
                nc.sync.dma_start(ctx_o[row0 : row0 + P, :], ob[:])

            def finalize(sbi, ctxa):
                for sc in range(NB // P):
                    finalize_sc(sbi, ctxa, sc)

            cur_emitter = qproj0
            pending = None  # (sbi, ctxa, {tci: ets}) for last two t-chunks
            for sbi in range(SBLK):
                mqh, mql = cur_emitter[0]
                next_emitter = (
                    make_qproj_emitter(sbi + 1) if sbi + 1 < SBLK else None
                )

                # finish the previous block: tail ctx matmuls + finalize
                if pending is not None:
                    p_sbi, p_ctxa, p_tail = pending
                    for tc_i in sorted(p_tail):
                        emit_ctx(p_ctxa, tc_i, p_tail[tc_i])
                    finalize(p_sbi, p_ctxa)
                    pending = None

                ctxa = [
                    psc.tile([P, HG, DH + 1], f32, tag="c", name=f"ctxa_{sbi}_{sc}")
                    for sc in range(NB // P)
                ]
                live_exp = {}
                for tci in range(TCH):
                    cur = []
                    for j in range(HG):
                        sp = psm.tile([P, NB], f32, tag="m")
                        score_mm(sp, mqh, mql, j, tci)
                        et = epool.tile([P, NB], f16, tag="et")
                        nc.scalar.activation(et, sp, Exp, scale=EXP_SCALE)
                        cur.append(et)
                    live_exp[tci] = cur
                    # ctx for tci-2: leaves slack for finalize of the
                    # previous block to release the psc banks
                    if tci >= 2:
                        emit_ctx(ctxa, tci - 2, live_exp.pop(tci - 2))
                    # one qproj step for block sbi+1 every other t-chunk
                    if next_emitter is not None and tci in (1, 3, 5, 7, 9, 11, 13, 14):
                        next_emitter[1](1)

                pending = (sbi, ctxa, live_exp)
                cur_emitter = next_emitter

            # last block: interleave the final t-chunk's ctx matmuls with
            # per-sc finalize so the normalize+store pipeline starts ASAP
            p_sbi, p_ctxa, p_tail = pending
            tcis = sorted(p_tail)
            for tc_i in tcis[:-1]:
                emit_ctx(p_ctxa, tc_i, p_tail[tc_i])
            last = tcis[-1]
            for sc in range(NB // P):
                emit_ctx(p_ctxa, last, p_tail[last], sc_list=[sc])
                finalize_sc(p_sbi, p_ctxa, sc)

    nc.compile()
    return nc


def make_in_maps(hidden_states, Wq, Wk, Wv, bv, mixing):
    """Host-side sharding: build per-core input dicts."""
    import ml_dtypes

    f8 = ml_dtypes.float8_e4m3
    hidden_states = np.asarray(hidden_states, dtype=np.float32)
    Wq = np.asarray(Wq, dtype=np.float32)
    Wk = np.asarray(Wk, dtype=np.float32)
    Wv = np.asarray(Wv, dtype=np.float32)
    bv = np.asarray(bv, dtype=np.float32)
    mixing = np.asarray(mixing, dtype=np.float32)

    def hilo(x):
        hi = np.ascontiguousarray(x).astype(f8)
        lo = (x - hi.astype(np.float32)).astype(f8)
        return hi, lo

    wqT = 32.0 * Wq.T  # [d, e]
    wkT = 32.0 * Wk.T
    ht_by_b = [hilo(hidden_states[b].T) for b in range(B)]

    wvT = 32.0 * Wv.T  # [d, dv]
    wvt_by_g = [hilo(wvT[:, g * HG * DH : (g + 1) * HG * DH]) for g in range(HG)]

    # per-group importance permutation of the e axis: sort by
    # sum_j mixing[j,e]^2 descending so low-importance e's land in the
    # term-pruned chunks.
    wq_by_g, wk_by_g, mix_by_g = [], [], []
    for g in range(HG):
        mrows = mixing[g * HG : (g + 1) * HG]  # [4, 1024]
        imp = (mrows**2).sum(axis=0)
        perm = np.argsort(-imp)
        wq_by_g.append(hilo(wqT[:, perm]))
        wk_by_g.append(hilo(wkT[:, perm]))
        mperm = mrows[:, perm]  # [4, 1024]
        # mix[p, e*HG + j] = mperm[j, e*128+p] / 4
        m = np.ascontiguousarray(
            mperm.reshape(HG, EC, P).transpose(2, 1, 0).reshape(P, EC * HG) / 4.0
        ).astype(np.float32)
        mix_by_g.append(m)

    in_maps = []
    for c in range(N_CORES):
        b, g = divmod(c, HG)
        in_maps.append(
            {
                "hthi": ht_by_b[b][0],
                "htlo": ht_by_b[b][1],
                "wqthi": wq_by_g[g][0],
                "wqtlo": wq_by_g[g][1],
                "wkthi": wk_by_g[g][0],
                "wktlo": wk_by_g[g][1],
                "wvthi": wvt_by_g[g][0],
                "wvtlo": wvt_by_g[g][1],
                "mix": mix_by_g[g],
            }
        )
    return in_maps


def assemble_output(results):
    """results: list of per-core dicts with 'ctx' [S, 256] f32. The v bias is
    added here: softmax rows sum to 1, so ctx = probs@v + bv."""
    out = np.empty((B, S, DV), dtype=np.float32)
    bv = _CACHE["bv"]
    for c in range(N_CORES):
        b, g = divmod(c, HG)
        sl = slice(g * HG * DH, (g + 1) * HG * DH)
        out[b, :, sl] = results[c]["ctx"] + bv[sl][None, :]
    return out


def _get_runner():
    """Build (once) a jitted shard_map over the 8 cores running the compiled
    Bass program via the bass_exec custom call."""
    if "runner" in _CACHE:
        return _CACHE["runner"]

    import jax
    import concourse.mybir as mybir
    from jax.sharding import Mesh, PartitionSpec
    from jax.experimental.shard_map import shard_map
    from concourse import bass2jax
    from concourse.bass2jax import _bass_exec_p, partition_id_tensor

    bass2jax.install_neuronx_cc_hook()
    nc = _CACHE.setdefault("nc", build_program())

    part_name = nc.partition_id_tensor.name if nc.partition_id_tensor else None
    dbg_name = nc.dbg_addr.name if nc.dbg_addr is not None else None
    in_names, out_names, out_avals, zero_outs = [], [], [], []
    for alloc in nc.m.functions[0].allocations:
        if not isinstance(alloc, mybir.MemoryLocationSet):
            continue
        name = alloc.memorylocations[0].name
        if alloc.kind == "ExternalInput":
            if name != part_name:
                in_names.append(name)
        elif alloc.kind == "ExternalOutput":
            out_names.append(name)
            shape = tuple(alloc.tensor_shape)
            dtype = mybir.dt.np(alloc.dtype)
            out_avals.append(jax.core.ShapedArray(shape, dtype))
            zero_outs.append(np.zeros(shape, dtype))
    n_params = len(in_names)
    all_names = in_names + out_names + ([part_name] if part_name else [])

    def _body(*args):
        operands = list(args)
        if part_name is not None:
            operands.append(partition_id_tensor())
        outs = _bass_exec_p.bind(
            *operands,
            out_avals=tuple(out_avals),
            in_names=tuple(all_names),
            out_names=tuple(out_names),
            lowering_input_output_aliases=(),
            sim_require_finite=True,
            sim_require_nnan=True,
            nc=nc,
        )
        return tuple(outs)

    devices = jax.devices()[:N_CORES]
    mesh = Mesh(np.asarray(devices), ("core",))
    spec = PartitionSpec("core")
    sharded = jax.jit(
        shard_map(
            _body,
            mesh=mesh,
            in_specs=(spec,) * (n_params + len(out_names)),
            out_specs=(spec,) * len(out_names),
            check_rep=False,
        ),
        keep_unused=True,
    )
    concat_zero = [
        np.zeros((N_CORES * z.shape[0], *z.shape[1:]), z.dtype) for z in zero_outs
    ]

    def run(in_maps):
        def core_input(c, name):
            if name == dbg_name:
                return np.zeros((1, 2), np.uint32)
            return in_maps[c][name]

        concat_in = [
            np.concatenate([core_input(c, name) for c in range(N_CORES)], axis=0)
            for name in in_names
        ]
        out_arrs = sharded(*concat_in, *concat_zero)
        return [
            {
                name: np.asarray(out_arrs[i]).reshape(
                    N_CORES, *out_avals[i].shape
                )[c]
                for i, name in enumerate(out_names)
            }
            for c in range(N_CORES)
        ]

    _CACHE["runner"] = run
    return run


def kernel(hidden_states, Wq, Wk, Wv, bv, mixing):
    run = _get_runner()
    _CACHE["bv"] = np.asarray(bv, dtype=np.float32)
    in_maps = make_in_maps(hidden_states, Wq, Wk, Wv, bv, mixing)
    return assemble_output(run(in_maps))
